# revision 1
# baseline (speedup 1.0000x reference)
"""Trainium2 Bass kernel for nn_CombinedRepeatCausalLinear (PE version).

Math: out[r, t] = sum_{s<=t} x[r, s] * (w0[s]*dv0^(t-s) + w1[t]*dv1^(t-s)) + bias[t]

Chunked linear-attention formulation (chunk L=126 along S):
  - Diagonal blocks D_c[s_l, t_l] (upper-triangular, [128,128] with 2 extra
    "reduction" columns producing decay-weighted chunk sums S0_c, S1_c in
    psum rows 126/127).
  - Cross-chunk contribution is rank-2 per source chunk:
      out[t] += sum_{c'<c(t)} dv0^(t-e_c')*S1_c'[r] + w1[t]*dv1^(t-e_c')*S0_c'[r]
    implemented as a second matmul per chunk against a host-built matrix.

All matmuls are K=128 (host matrices zero-padded) so the PE array stays
fully active and the HAM clock-gate holds the warm 2.4 GHz state; fp32
throughout (HI/LO 2-pass, exact fp32 products). The output is computed
transposed (t on partitions) so the host-built matrices are always the
stationary operand; the host ships x pre-transposed per shard and
transposes the gathered result back.

Data-parallel across 8 NeuronCores on the fused B*E axis.
"""

import sys

if "/opt/trn_rl_repo" not in sys.path:
    sys.path.insert(0, "/opt/trn_rl_repo")

import numpy as np

import concourse.mybir as mybir
from concourse import bacc
from concourse.bass_utils import run_bass_kernel_spmd
from concourse.mybir import AluOpType
from concourse.tile import TileContext

_P = 128
_B, _E, _S = 4, 2048, 2048
_NCORES = 8
_R = (_B * _E) // _NCORES  # 1024 rows (r) per core
_L = 126  # chunk length along S
_NCH = (_S + _L - 1) // _L  # 17 chunks (last has 32)
_HALF = 512  # r per matmul (one PSUM bank, fp32)
_NH = _R // _HALF  # 2 halves

_F32 = mybir.dt.float32


def _chunk_len(c):
    return min(_L, _S - c * _L)


def _build_host_mats(w0, w1, dv0, dv1, bias, with_bias):
    """Build D [128, NCH*128] and M [128, NCH*128] in float64, cast f32."""
    w0 = w0.astype(np.float64)
    w1 = w1.astype(np.float64)
    D = np.zeros((_NCH * _P, _P), dtype=np.float64)
    for c in range(_NCH):
        Lc = _chunk_len(c)
        base = c * _L
        sl = np.arange(Lc)
        tl = np.arange(Lc)
        diff = tl[None, :] - sl[:, None]
        mask = diff >= 0
        blk = np.where(
            mask,
            w0[base + sl][:, None] * (dv0 ** np.maximum(diff, 0))
            + w1[base + tl][None, :] * (dv1 ** np.maximum(diff, 0)),
            0.0,
        )
        Db = D[c * _P : (c + 1) * _P]
        Db[:Lc, :Lc] = blk
        # reduction columns: col 126 -> S0_c (dv1-weighted sum),
        #                    col 127 -> S1_c (w0*dv0-weighted sum)
        Db[:Lc, 126] = dv1 ** (Lc - 1 - sl)
        Db[:Lc, 127] = w0[base + sl] * dv0 ** (Lc - 1 - sl)

    off = 1 if with_bias else 0
    # M padded to 128 contraction rows (rows >= off+2*NCH are zero)
    M = np.zeros((_P, _NCH * _P), dtype=np.float64)
    for c in range(_NCH):
        Lc = _chunk_len(c)
        t = c * _L + np.arange(Lc)
        if with_bias:
            M[0, c * _P : c * _P + Lc] = bias.astype(np.float64)[t]
        for cp in range(c):
            e_cp = cp * _L + _chunk_len(cp) - 1
            M[off + 2 * cp, c * _P : c * _P + Lc] = w1[t] * (dv1 ** (t - e_cp))
            M[off + 2 * cp + 1, c * _P : c * _P + Lc] = dv0 ** (t - e_cp)
    return D.astype(np.float32), M.astype(np.float32)


def _build(with_bias):
    off = 1 if with_bias else 0
    nc = bacc.Bacc(
        "TRN2",
        target_bir_lowering=False,
        debug=False,
        enable_asserts=False,
        num_devices=_NCORES,
    )
    xt = nc.dram_tensor("xt", [_S, _R], _F32, kind="ExternalInput").ap()
    Dd = nc.dram_tensor("Dd", [_NCH * _P, _P], _F32, kind="ExternalInput").ap()
    Md = nc.dram_tensor("Md", [_P, _NCH * _P], _F32, kind="ExternalInput").ap()
    outT = nc.dram_tensor("outT", [_S, _R], _F32, kind="ExternalOutput").ap()

    with TileContext(nc) as tc:
        with (
            tc.tile_pool(name="consts", bufs=1) as cpool,
            tc.tile_pool(name="xin", bufs=8) as xpool,
            tc.tile_pool(name="dg", bufs=1) as dgpool,
            tc.tile_pool(name="ot", bufs=4) as otpool,
            tc.tile_pool(name="pd", bufs=5, space="PSUM") as pdpool,
            tc.tile_pool(name="po", bufs=3, space="PSUM") as popool,
        ):
            sall = cpool.tile([_P, _R], _F32)
            nc.gpsimd.memset(sall[:], 0.0)
            if with_bias:
                nc.gpsimd.memset(sall[0:1, :], 1.0)
            # dedicated last-chunk x tile, zero-filled up front so the
            # memset is off the phase-1 critical path (K=128 contraction
            # reads the zero tail rows)
            xlast = cpool.tile([_P, _R], _F32)
            nc.gpsimd.memset(xlast[:], 0.0)

            # ---- Phase 1: diagonal blocks + chunk reductions ----
            Dt = None
            Mt = None
            dg_tiles = []
            for c in range(_NCH):
                Lc = _chunk_len(c)
                rows = min(_P, _S - c * _L)  # 128, except last chunk: 32
                if rows < _P:
                    xtile = xlast
                else:
                    xtile = xpool.tile([_P, _R], _F32, tag="x", name="x")
                nc.sync.dma_start(xtile[:rows, :], xt[c * _L : c * _L + rows, :])
                dtile = xpool.tile([_P, _P], _F32, tag="d", name="d")
                nc.scalar.dma_start(dtile[:], Dd[c * _P : (c + 1) * _P, :])
                if c == 8:
                    # M is only needed for phase 2; load it mid-phase
                    Mt = cpool.tile([_P, _NCH * _P], _F32)
                    nc.scalar.dma_start(Mt[:], Md[:])
                dg = dgpool.tile([_P, _R], _F32, tag=f"dg{c}", name="dg")
                for h in range(_NH):
                    pd = pdpool.tile([_P, _HALF], _F32, tag="pd", name="pd")
                    nc.tensor.matmul(
                        pd[:],
                        dtile[:],
                        xtile[:, h * _HALF : (h + 1) * _HALF],
                        start=True,
                        stop=True,
                    )
                    nc.vector.tensor_copy(dg[:, h * _HALF : (h + 1) * _HALF], pd[:])
                    # move the chunk-sum rows into Sall partitions (2c, 2c+1)
                    nc.gpsimd.dma_start(
                        sall[off + 2 * c : off + 2 * c + 2, h * _HALF : (h + 1) * _HALF],
                        dg[126:128, h * _HALF : (h + 1) * _HALF],
                    )
                dg_tiles.append(dg)

            # ---- Phase 2: cross-chunk offsets + combine + store ----
            for c in range(_NCH):
                Lc = _chunk_len(c)
                dg = dg_tiles[c]
                if c == 0 and not with_bias:
                    nc.sync.dma_start(outT[0:_L, :], dg[:_L, :])
                    continue
                ot = otpool.tile([_P, _R], _F32, tag="ot", name="ot")
                for h in range(_NH):
                    po = popool.tile([_P, _HALF], _F32, tag="po", name="po")
                    nc.tensor.matmul(
                        po[:],
                        Mt[:, c * _P : (c + 1) * _P],
                        sall[:, h * _HALF : (h + 1) * _HALF],
                        start=True,
                        stop=True,
                    )
                    nc.vector.tensor_tensor(
                        ot[:, h * _HALF : (h + 1) * _HALF],
                        dg[:, h * _HALF : (h + 1) * _HALF],
                        po[:],
                        AluOpType.add,
                    )
                eng = nc.sync if c % 2 == 0 else nc.scalar
                eng.dma_start(outT[c * _L : c * _L + Lc, :], ot[:Lc, :])
    nc.compile()
    return nc


def _run(x, weight, bias, decay_value, trace=False):
    x = np.asarray(x, dtype=np.float32)
    w = np.asarray(weight, dtype=np.float32)
    b = np.asarray(bias, dtype=np.float32)
    dv = np.asarray(decay_value, dtype=np.float32)
    dv0 = float(np.clip(dv[0, 0], 0.9, 1.0))
    dv1 = float(np.clip(dv[1, 0], 0.9, 1.0))
    with_bias = bool(np.any(b))

    D, M = _build_host_mats(w[0], w[1], dv0, dv1, b, with_bias)
    nc = _build(with_bias)

    xf = x.reshape(_B * _E, _S)
    xT = np.ascontiguousarray(xf.T)  # [S, B*E]
    in_maps = []
    for c in range(_NCORES):
        in_maps.append(
            {
                "xt": np.ascontiguousarray(xT[:, c * _R : (c + 1) * _R]),
                "Dd": D,
                "Md": M,
            }
        )

    res = run_bass_kernel_spmd(nc, in_maps, core_ids=list(range(_NCORES)), trace=trace)
    outT = np.concatenate(
        [res.results[c]["outT"] for c in range(_NCORES)], axis=1
    )  # [S, B*E]
    full = np.ascontiguousarray(outT.T).reshape(_B, _E, _S)
    return full, res


def kernel(x, weight, bias, decay_value):
    full, _ = _run(x, weight, bias, decay_value, trace=False)
    return full



# revision 2
# speedup vs baseline: 1.2565x; 1.2565x over previous
"""Trainium2 Bass kernel for nn_CombinedRepeatCausalLinear (bf16 version).

Math: out[r, t] = sum_{s<=t} x[r, s] * (w0[s]*dv0^(t-s) + w1[t]*dv1^(t-s)) + bias[t]

Chunked linear-attention formulation (chunk L=126 along S):
  - Diagonal blocks D_c[s_l, t_l] (upper-triangular, [128,128] with 2 extra
    "reduction" columns producing decay-weighted chunk sums S0_c, S1_c in
    rows 126/127 of the diagonal result).
  - Cross-chunk contribution is rank-2 per source chunk:
      out[t] += w1[t]*dv1^(t-e_c')*S0_c'[r] + dv0^(t-e_c')*S1_c'[r]
    implemented as a second matmul per chunk against a host-built matrix M
    whose contraction rows hold the per-chunk sums (gathered in `sall`).

Everything on the wire and in SBUF is bf16 (tolerance is 2e-2; bf16
end-to-end gives ~3e-3): halves HBM traffic vs fp32 and makes every
matmul single-pass (fp32 needs HI/LO 2-pass on the PE).  PSUM stays fp32.

Engine assignment (each FD=1024 chunk-op in one instruction):
  sync   : x chunk loads only (prefetch never blocked by output deps)
  scalar : D/M loads, PSUM->SBUF copies (ACT activation-copy), output stores
  vector : cross+diag merge adds (TENSOR_TENSOR, PSUM operand)
  gpsimd : chunk-sum extraction SBUF->SBUF DMAs, sall bias memset
  tensor : 2 diag + 2 cross matmuls per chunk (N=512, K<=128, bf16)

Phase-2 for chunk c-2 is emitted inside chunk c's loop iteration so the
vector/scalar/DMA work overlaps the input stream instead of trailing it.

Data-parallel across 8 NeuronCores on the fused B*E axis.
"""

import sys

if "/opt/trn_rl_repo" not in sys.path:
    sys.path.insert(0, "/opt/trn_rl_repo")

import ml_dtypes
import numpy as np

import concourse.mybir as mybir
from concourse import bacc
from concourse.bass_utils import run_bass_kernel_spmd
from concourse.mybir import AluOpType
from concourse.tile import TileContext

_P = 128
_B, _E, _S = 4, 2048, 2048
_NCORES = 8
_R = (_B * _E) // _NCORES  # 1024 rows (r) per core
_L = 126  # chunk length along S
_NCH = (_S + _L - 1) // _L  # 17 chunks (last has 32)
_HALF = 512  # r per matmul (one PSUM bank, fp32)
_XROWS = _NCH * _P  # 2176 padded x rows (>= 126*16+128)
_OROWS = _NCH * _P  # padded out rows (>= 126*17)

_F32 = mybir.dt.float32
_BF16 = mybir.dt.bfloat16
_npbf = ml_dtypes.bfloat16


def _chunk_len(c):
    return min(_L, _S - c * _L)


def _build_host_mats(w0, w1, dv0, dv1, bias, with_bias):
    """Build D [NCH*128, 128] and M [srows, NCH*128] in float64, cast bf16."""
    w0 = w0.astype(np.float64)
    w1 = w1.astype(np.float64)
    D = np.zeros((_NCH * _P, _P), dtype=np.float64)
    for c in range(_NCH):
        Lc = _chunk_len(c)
        base = c * _L
        sl = np.arange(Lc)
        tl = np.arange(Lc)
        diff = tl[None, :] - sl[:, None]
        mask = diff >= 0
        blk = np.where(
            mask,
            w0[base + sl][:, None] * (dv0 ** np.maximum(diff, 0))
            + w1[base + tl][None, :] * (dv1 ** np.maximum(diff, 0)),
            0.0,
        )
        Db = D[c * _P : (c + 1) * _P]
        Db[:Lc, :Lc] = blk
        # reduction columns: col 126 -> S0_c (dv1-weighted sum),
        #                    col 127 -> S1_c (w0*dv0-weighted sum)
        Db[:Lc, 126] = dv1 ** (Lc - 1 - sl)
        Db[:Lc, 127] = w0[base + sl] * dv0 ** (Lc - 1 - sl)

    off = 1 if with_bias else 0
    srows = off + 2 * _NCH
    M = np.zeros((srows, _NCH * _P), dtype=np.float64)
    for c in range(_NCH):
        Lc = _chunk_len(c)
        t = c * _L + np.arange(Lc)
        if with_bias:
            M[0, c * _P : c * _P + Lc] = bias.astype(np.float64)[t]
        for cp in range(c):
            e_cp = cp * _L + _chunk_len(cp) - 1
            M[off + 2 * cp, c * _P : c * _P + Lc] = w1[t] * (dv1 ** (t - e_cp))
            M[off + 2 * cp + 1, c * _P : c * _P + Lc] = dv0 ** (t - e_cp)
    return D.astype(_npbf), M.astype(_npbf)


def _build(with_bias):
    off = 1 if with_bias else 0
    srows = off + 2 * _NCH
    nc = bacc.Bacc(
        "TRN2",
        target_bir_lowering=False,
        debug=False,
        enable_asserts=False,
        num_devices=_NCORES,
    )
    xt = nc.dram_tensor("xt", [_XROWS, _R], _BF16, kind="ExternalInput").ap()
    Dd = nc.dram_tensor("Dd", [_NCH * _P, _P], _BF16, kind="ExternalInput").ap()
    Md = nc.dram_tensor("Md", [srows, _NCH * _P], _BF16, kind="ExternalInput").ap()
    outT = nc.dram_tensor("outT", [_OROWS, _R], _BF16, kind="ExternalOutput").ap()

    with TileContext(nc) as tc:
        with (
            tc.tile_pool(name="consts", bufs=1) as cpool,
            tc.tile_pool(name="xin", bufs=6) as xpool,
            tc.tile_pool(name="dg", bufs=6) as dgpool,
            tc.tile_pool(name="pd", bufs=2, space="PSUM") as pdpool,
            tc.tile_pool(name="po", bufs=2, space="PSUM") as popool,
        ):
            # stationary matrices, loaded up-front on the scalar queue
            Dt = cpool.tile([_P, _NCH * _P], _BF16)
            for c in range(_NCH):
                nc.scalar.dma_start(
                    Dt[:, c * _P : (c + 1) * _P], Dd[c * _P : (c + 1) * _P, :]
                )
            Mt = cpool.tile([srows, _NCH * _P], _BF16)
            nc.scalar.dma_start(Mt[:], Md[:])
            sall = cpool.tile([srows, _R], _BF16)
            if with_bias:
                nc.gpsimd.memset(sall[0:1, :], 1.0)

            dg_tiles = {}

            def phase2(j):
                dg = dg_tiles.pop(j)
                kj = off + 2 * j
                if kj > 0:
                    po = popool.tile([_P, _R], _F32, tag="po", name="po")
                    for h in range(2):
                        nc.tensor.matmul(
                            po[:, h * _HALF : (h + 1) * _HALF],
                            Mt[0:kj, j * _P : (j + 1) * _P],
                            sall[0:kj, h * _HALF : (h + 1) * _HALF],
                            start=True,
                            stop=True,
                        )
                    nc.vector.tensor_tensor(
                        dg[0:_L, :], dg[0:_L, :], po[0:_L, :], AluOpType.add
                    )
                nc.scalar.dma_start(outT[j * _L : (j + 1) * _L, :], dg[0:_L, :])

            for c in range(_NCH):
                xtile = xpool.tile([_P, _R], _BF16, tag="x", name="x")
                nc.sync.dma_start(xtile[:], xt[c * _L : c * _L + _P, :])
                pd = pdpool.tile([_P, _R], _F32, tag="pd", name="pd")
                for h in range(2):
                    nc.tensor.matmul(
                        pd[:, h * _HALF : (h + 1) * _HALF],
                        Dt[:, c * _P : (c + 1) * _P],
                        xtile[:, h * _HALF : (h + 1) * _HALF],
                        start=True,
                        stop=True,
                    )
                dg = dgpool.tile([_P, _R], _BF16, tag="dg", name="dg")
                nc.scalar.copy(dg[:], pd[:])
                dg_tiles[c] = dg
                if c < _NCH - 1:
                    nc.gpsimd.dma_start(
                        sall[off + 2 * c : off + 2 * c + 2, :], dg[126:128, :]
                    )
                if c >= 2:
                    phase2(c - 2)
            phase2(_NCH - 2)
            phase2(_NCH - 1)
    nc.compile()
    return nc


def _run(x, weight, bias, decay_value, trace=False):
    x = np.asarray(x, dtype=np.float32)
    w = np.asarray(weight, dtype=np.float32)
    b = np.asarray(bias, dtype=np.float32)
    dv = np.asarray(decay_value, dtype=np.float32)
    dv0 = float(np.clip(dv[0, 0], 0.9, 1.0))
    dv1 = float(np.clip(dv[1, 0], 0.9, 1.0))
    with_bias = bool(np.any(b))

    D, M = _build_host_mats(w[0], w[1], dv0, dv1, b, with_bias)
    nc = _build(with_bias)

    xf = x.reshape(_B * _E, _S)
    xTb = np.zeros((_XROWS, _B * _E), dtype=_npbf)
    xTb[:_S] = xf.T.astype(_npbf)
    in_maps = []
    for c in range(_NCORES):
        in_maps.append(
            {
                "xt": np.ascontiguousarray(xTb[:, c * _R : (c + 1) * _R]),
                "Dd": D,
                "Md": M,
            }
        )

    res = run_bass_kernel_spmd(nc, in_maps, core_ids=list(range(_NCORES)), trace=trace)
    outT = np.concatenate(
        [np.asarray(res.results[c]["outT"]) for c in range(_NCORES)], axis=1
    )  # [_OROWS, B*E] bf16
    full = np.ascontiguousarray(outT[:_S].T, dtype=np.float32).reshape(_B, _E, _S)
    return full, res


def kernel(x, weight, bias, decay_value):
    full, _ = _run(x, weight, bias, decay_value, trace=False)
    return full


# revision 7
# speedup vs baseline: 1.6408x; 1.3059x over previous
"""Trainium2 Bass kernel for nn_CombinedRepeatCausalLinear (bf16 version).

Math: out[r, t] = sum_{s<=t} x[r, s] * (w0[s]*dv0^(t-s) + w1[t]*dv1^(t-s)) + bias[t]

Chunked linear-attention formulation (chunk L=126 along S):
  - Diagonal blocks D_c[s_l, t_l] (upper-triangular, [128,128] with 2 extra
    "reduction" columns producing decay-weighted chunk sums S0_c, S1_c in
    rows 126/127 of the diagonal result).
  - Cross-chunk contribution is rank-2 per source chunk:
      out[t] += w1[t]*dv1^(t-e_c')*S0_c'[r] + dv0^(t-e_c')*S1_c'[r]
    implemented as a second matmul per chunk against a host-built matrix M
    whose contraction rows hold the per-chunk sums (gathered in `sall`).

Everything on the wire and in SBUF is bf16 (tolerance is 2e-2; bf16
end-to-end gives ~3e-3): halves HBM traffic vs fp32 and makes every
matmul single-pass (fp32 needs HI/LO 2-pass on the PE).  PSUM stays fp32.

Engine assignment (each FD=1024 chunk-op in one instruction):
  sync   : x chunk loads only (prefetch never blocked by output deps)
  scalar : D/M loads, PSUM->SBUF copies (ACT activation-copy), output stores
  vector : cross+diag merge adds (TENSOR_TENSOR, PSUM operand)
  gpsimd : chunk-sum extraction SBUF->SBUF DMAs, sall bias memset
  tensor : 2 diag + 2 cross matmuls per chunk (N=512, K<=128, bf16)

Phase-2 for chunk c-2 is emitted inside chunk c's loop iteration so the
vector/scalar/DMA work overlaps the input stream instead of trailing it.

Data-parallel across 8 NeuronCores on the fused B*E axis.
"""

import sys

if "/opt/trn_rl_repo" not in sys.path:
    sys.path.insert(0, "/opt/trn_rl_repo")

import ml_dtypes
import numpy as np

import concourse.mybir as mybir
from concourse import bacc
from concourse.bass_utils import run_bass_kernel_spmd
from concourse.mybir import AluOpType
from concourse.tile import TileContext

_P = 128
_B, _E, _S = 4, 2048, 2048
_NCORES = 8
_R = (_B * _E) // _NCORES  # 1024 rows (r) per core
_L = 126  # chunk length along S
_NCH = (_S + _L - 1) // _L  # 17 chunks (last has 32)
_HALF = 512  # r per matmul (one PSUM bank, fp32)
_XROWS = _NCH * _P  # 2176 padded x rows (>= 126*16+128)
_OROWS = _NCH * _P  # padded out rows (>= 126*17)

_F32 = mybir.dt.float32
_BF16 = mybir.dt.bfloat16
_npbf = ml_dtypes.bfloat16


def _chunk_len(c):
    return min(_L, _S - c * _L)


def _build_host_mats(w0, w1, dv0, dv1, bias, with_bias):
    """Build D [128, NCH*128] (partition-major, SBUF layout: one line-rate DMA)
    and M [srows, NCH*128] in float64, cast bf16."""
    w0 = w0.astype(np.float64)
    w1 = w1.astype(np.float64)
    D = np.zeros((_NCH * _P, _P), dtype=np.float64)
    for c in range(_NCH):
        Lc = _chunk_len(c)
        base = c * _L
        sl = np.arange(Lc)
        tl = np.arange(Lc)
        diff = tl[None, :] - sl[:, None]
        mask = diff >= 0
        blk = np.where(
            mask,
            w0[base + sl][:, None] * (dv0 ** np.maximum(diff, 0))
            + w1[base + tl][None, :] * (dv1 ** np.maximum(diff, 0)),
            0.0,
        )
        Db = D[c * _P : (c + 1) * _P]
        Db[:Lc, :Lc] = blk
        # reduction columns: col 126 -> S0_c (dv1-weighted sum),
        #                    col 127 -> S1_c (w0*dv0-weighted sum)
        Db[:Lc, 126] = dv1 ** (Lc - 1 - sl)
        Db[:Lc, 127] = w0[base + sl] * dv0 ** (Lc - 1 - sl)

    off = 1 if with_bias else 0
    srows = off + 2 * _NCH
    M = np.zeros((srows, _NCH * _P), dtype=np.float64)
    for c in range(_NCH):
        Lc = _chunk_len(c)
        t = c * _L + np.arange(Lc)
        if with_bias:
            M[0, c * _P : c * _P + Lc] = bias.astype(np.float64)[t]
        for cp in range(c):
            e_cp = cp * _L + _chunk_len(cp) - 1
            M[off + 2 * cp, c * _P : c * _P + Lc] = w1[t] * (dv1 ** (t - e_cp))
            M[off + 2 * cp + 1, c * _P : c * _P + Lc] = dv0 ** (t - e_cp)
    # partition-major D: D_sb[p, c*128 + t] = D[c*128 + p, t]
    D_sb = np.ascontiguousarray(
        D.reshape(_NCH, _P, _P).transpose(1, 0, 2).reshape(_P, _NCH * _P)
    )
    return D_sb.astype(_npbf), M.astype(_npbf)


def _build(with_bias):
    off = 1 if with_bias else 0
    srows = off + 2 * _NCH
    nc = bacc.Bacc(
        "TRN2",
        target_bir_lowering=False,
        debug=False,
        enable_asserts=False,
        num_devices=_NCORES,
    )
    xt = nc.dram_tensor("xt", [_XROWS, _R], _BF16, kind="ExternalInput").ap()
    Dd = nc.dram_tensor("Dd", [_P, _NCH * _P], _BF16, kind="ExternalInput").ap()
    Md = nc.dram_tensor("Md", [srows, _NCH * _P], _BF16, kind="ExternalInput").ap()
    outT = nc.dram_tensor("outT", [_OROWS, _R], _BF16, kind="ExternalOutput").ap()

    with TileContext(nc) as tc:
        with (
            tc.tile_pool(name="consts", bufs=1) as cpool,
            tc.tile_pool(name="xin", bufs=12) as xpool,
            tc.tile_pool(name="dg", bufs=8) as dgpool,
            tc.tile_pool(name="pd", bufs=2, space="PSUM") as pdpool,
            tc.tile_pool(name="po", bufs=2, space="PSUM") as popool,
        ):
            # stationary matrices: single line-rate DMAs on the (otherwise
            # idle-at-start) gpsimd queue, keeping scalar free for copies
            Dt = cpool.tile([_P, _NCH * _P], _BF16)
            nc.gpsimd.dma_start(Dt[:], Dd[:])
            Mt = cpool.tile([srows, _NCH * _P], _BF16)
            nc.gpsimd.dma_start(Mt[:], Md[:])
            sall = cpool.tile([srows, _R], _BF16)
            if with_bias:
                nc.gpsimd.memset(sall[0:1, :], 1.0)

            # PE warm-up: ~12 back-to-back dummy matmuls (~6us) flip the HAM
            # clock gate to 8/8 (2.4 GHz) during the DMA ramp, so every real
            # matmul runs warm.  They reuse the po PSUM ring (free until
            # phase2(0)) and read a small memset tile.
            wsrc = cpool.tile([_P, _HALF], _BF16)
            nc.vector.memset(wsrc[:], 0.0)
            warm = popool.tile([_P, _R], _F32, tag="po", name="warm")
            for _ in range(12):
                nc.tensor.matmul(
                    warm[:, 0:_HALF],
                    wsrc[:, 0:_P],
                    wsrc[:],
                    start=True,
                    stop=True,
                )

            dg_tiles = {}

            def phase2(j):
                dg = dg_tiles.pop(j)
                kj = off + 2 * j
                if kj > 0:
                    po = popool.tile([_P, _R], _F32, tag="po", name="po")
                    for h in range(2):
                        nc.tensor.matmul(
                            po[:, h * _HALF : (h + 1) * _HALF],
                            Mt[0:kj, j * _P : (j + 1) * _P],
                            sall[0:kj, h * _HALF : (h + 1) * _HALF],
                            start=True,
                            stop=True,
                        )
                    nc.vector.tensor_tensor(
                        dg[0:_L, :], dg[0:_L, :], po[0:_L, :], AluOpType.add
                    )
                eng = nc.sync if j % 2 == 0 else nc.gpsimd
                eng.dma_start(outT[j * _L : (j + 1) * _L, :], dg[0:_L, :])

            for c in range(_NCH):
                xtile = xpool.tile([_P, _R], _BF16, tag="x", name="x")
                nc.sync.dma_start(xtile[:], xt[c * _L : c * _L + _P, :])
                pd = pdpool.tile([_P, _R], _F32, tag="pd", name="pd")
                for h in range(2):
                    nc.tensor.matmul(
                        pd[:, h * _HALF : (h + 1) * _HALF],
                        Dt[:, c * _P : (c + 1) * _P],
                        xtile[:, h * _HALF : (h + 1) * _HALF],
                        start=True,
                        stop=True,
                    )
                dg = dgpool.tile([_P, _R], _BF16, tag="dg", name="dg")
                nc.scalar.copy(dg[:], pd[:])
                dg_tiles[c] = dg
                if c < _NCH - 1:
                    nc.gpsimd.dma_start(
                        sall[off + 2 * c : off + 2 * c + 2, :], dg[126:128, :]
                    )
                if c >= 2:
                    phase2(c - 2)
            phase2(_NCH - 2)
            phase2(_NCH - 1)
    nc.compile()
    return nc


def _run(x, weight, bias, decay_value, trace=False):
    x = np.asarray(x, dtype=np.float32)
    w = np.asarray(weight, dtype=np.float32)
    b = np.asarray(bias, dtype=np.float32)
    dv = np.asarray(decay_value, dtype=np.float32)
    dv0 = float(np.clip(dv[0, 0], 0.9, 1.0))
    dv1 = float(np.clip(dv[1, 0], 0.9, 1.0))
    with_bias = bool(np.any(b))

    D, M = _build_host_mats(w[0], w[1], dv0, dv1, b, with_bias)
    nc = _build(with_bias)

    xf = x.reshape(_B * _E, _S)
    xTb = np.zeros((_XROWS, _B * _E), dtype=_npbf)
    xTb[:_S] = xf.T.astype(_npbf)
    in_maps = []
    for c in range(_NCORES):
        in_maps.append(
            {
                "xt": np.ascontiguousarray(xTb[:, c * _R : (c + 1) * _R]),
                "Dd": D,
                "Md": M,
            }
        )

    res = run_bass_kernel_spmd(nc, in_maps, core_ids=list(range(_NCORES)), trace=trace)
    outT = np.concatenate(
        [np.asarray(res.results[c]["outT"]) for c in range(_NCORES)], axis=1
    )  # [_OROWS, B*E] bf16
    full = np.ascontiguousarray(outT[:_S].T, dtype=np.float32).reshape(_B, _E, _S)
    return full, res


def kernel(x, weight, bias, decay_value):
    full, _ = _run(x, weight, bias, decay_value, trace=False)
    return full
